# revision 1
# baseline (speedup 1.0000x reference)
"""Performer (FAVOR+) attention on 8 trn2 NeuronCores.

Sharding: tensor-parallel over the 16 heads (2 heads / core). Each core
applies its 128-column slice of the QKV projections to the replicated X,
runs FAVOR+ linear attention for its 2 heads, and produces its 128-column
slice of the output; the host concatenates the 8 slices.
"""
import numpy as np
import jax
import jax.numpy as jnp

B, S, D = 4, 4096, 1024
H = 16
HD = 64          # head dim
M = 256          # nb random features
N_CORES = 8
HPC = H // N_CORES          # heads per core = 2
COLS = HPC * HD             # projection columns per core = 128


def _per_core(X, mask, Wq, bq, Wk, bk, Wv, bv, proj):
    # X:[B,S,D] mask:[B,S] W*:[D,COLS] b*:[COLS] proj:[M,HD]
    Q = X @ Wq + bq
    K = X @ Wk + bk
    V = X @ Wv + bv

    def split(x):  # [B,S,COLS] -> [B,HPC,S,HD]
        return x.reshape(B, S, HPC, HD).transpose(0, 2, 1, 3)

    Q, K, V = split(Q), split(K), split(V)
    scale = HD ** -0.25
    m4 = mask[:, None, :, None]
    Qs = Q * scale
    Ks = K * scale * m4
    Vs = V * m4

    ratio = M ** -0.5

    def feat(x, is_query):
        x = x * (HD ** -0.25)
        u = jnp.einsum('bhsd,md->bhsm', x, proj)
        diag = 0.5 * jnp.sum(x * x, axis=-1, keepdims=True)
        if is_query:
            stab = jnp.max(u, axis=-1, keepdims=True)
        else:
            stab = jnp.max(u, axis=(-1, -2), keepdims=True)
        return ratio * (jnp.exp(u - diag - stab) + 1e-4)

    q_prime = feat(Qs, True)    # [B,HPC,S,M]
    k_prime = feat(Ks, False)
    kv = jnp.einsum('bhsm,bhsd->bhmd', k_prime, Vs)
    z = 1.0 / (jnp.einsum('bhsm,bhm->bhs', q_prime, jnp.sum(k_prime, axis=2)) + 1e-6)
    out = jnp.einsum('bhsm,bhmd->bhsd', q_prime, kv) * z[..., None]
    return out.transpose(0, 2, 1, 3).reshape(B, S, COLS)


_pmapped = None


def kernel(X, mask, Wq, bq, Wk, bk, Wv, bv, proj):
    global _pmapped
    devs = jax.devices()[:N_CORES]
    if _pmapped is None:
        _pmapped = jax.pmap(_per_core, devices=devs)

    def shard_cols(W):
        return np.stack([np.asarray(W[:, i * COLS:(i + 1) * COLS]) for i in range(N_CORES)])

    def shard_bias(b):
        return np.stack([np.asarray(b[i * COLS:(i + 1) * COLS]) for i in range(N_CORES)])

    rep = lambda a: np.broadcast_to(np.asarray(a), (N_CORES,) + np.asarray(a).shape)

    outs = _pmapped(
        rep(X), rep(mask),
        shard_cols(Wq), shard_bias(bq),
        shard_cols(Wk), shard_bias(bk),
        shard_cols(Wv), shard_bias(bv),
        rep(proj),
    )
    outs = np.asarray(outs)  # [8,B,S,COLS]; core i -> output cols i*128:(i+1)*128
    return np.concatenate(list(outs), axis=-1).astype(np.float32)


# revision 2
# speedup vs baseline: 1.1638x; 1.1638x over previous
"""Performer (FAVOR+) attention on 8 trn2 NeuronCores.

Sharding: tensor-parallel over the 16 heads. Primary path uses jit+GSPMD
with X replicated via a single device_put and the QKV projections
column-sharded (2 heads / core), so every op partitions head-locally with
no collectives. Falls back to an equivalent jax.pmap implementation if
the GSPMD path fails.
"""
import numpy as np
import jax
import jax.numpy as jnp
from jax.sharding import Mesh, NamedSharding, PartitionSpec as P

B, S, D = 4, 4096, 1024
H = 16
HD = 64          # head dim
M = 256          # nb random features
N_CORES = 8
HPC = H // N_CORES          # heads per core = 2
COLS = HPC * HD             # projection columns per core = 128


def _feat(x, proj, is_query):
    ratio = M ** -0.5
    x = x * (HD ** -0.25)
    u = jnp.einsum('bhsd,md->bhsm', x, proj)
    diag = 0.5 * jnp.sum(x * x, axis=-1, keepdims=True)
    if is_query:
        stab = jnp.max(u, axis=-1, keepdims=True)
    else:
        stab = jnp.max(u, axis=(-1, -2), keepdims=True)
    return ratio * (jnp.exp(u - diag - stab) + 1e-4)


def _attn(Q, K, V, mask, proj):
    # Q,K,V: [B,h,S,HD] for any number of heads h
    scale = HD ** -0.25
    m4 = mask[:, None, :, None]
    Qs = Q * scale
    Ks = K * scale * m4
    Vs = V * m4
    q_prime = _feat(Qs, proj, True)
    k_prime = _feat(Ks, proj, False)
    kv = jnp.einsum('bhsm,bhsd->bhmd', k_prime, Vs)
    z = 1.0 / (jnp.einsum('bhsm,bhm->bhs', q_prime, jnp.sum(k_prime, axis=2)) + 1e-6)
    return jnp.einsum('bhsm,bhmd->bhsd', q_prime, kv) * z[..., None]


# ----- primary: jit + GSPMD, full-shape math, head-sharded via weight cols -----

def _compute_full(X, mask, Wq, bq, Wk, bk, Wv, bv, proj):
    def split(x):  # [B,S,D] -> [B,H,S,HD]
        return x.reshape(B, S, H, HD).transpose(0, 2, 1, 3)

    Q = split(X @ Wq + bq)
    K = split(X @ Wk + bk)
    V = split(X @ Wv + bv)
    out = _attn(Q, K, V, mask, proj)          # [B,H,S,HD]
    return out.transpose(0, 2, 1, 3).reshape(B, S, D)


_gspmd = None


def _run_gspmd(X, mask, Wq, bq, Wk, bk, Wv, bv, proj):
    global _gspmd
    devs = jax.devices()[:N_CORES]
    mesh = Mesh(np.array(devs), ('x',))
    rep = NamedSharding(mesh, P())
    col = NamedSharding(mesh, P(None, 'x'))
    vec = NamedSharding(mesh, P('x'))
    outsh = NamedSharding(mesh, P(None, None, 'x'))
    if _gspmd is None:
        _gspmd = jax.jit(
            _compute_full,
            in_shardings=(rep, rep, col, vec, col, vec, col, vec, rep),
            out_shardings=outsh,
        )
    args = (
        jax.device_put(np.asarray(X, np.float32), rep),
        jax.device_put(np.asarray(mask, np.float32), rep),
        jax.device_put(np.asarray(Wq, np.float32), col),
        jax.device_put(np.asarray(bq, np.float32), vec),
        jax.device_put(np.asarray(Wk, np.float32), col),
        jax.device_put(np.asarray(bk, np.float32), vec),
        jax.device_put(np.asarray(Wv, np.float32), col),
        jax.device_put(np.asarray(bv, np.float32), vec),
        jax.device_put(np.asarray(proj, np.float32), rep),
    )
    out = _gspmd(*args)
    return np.asarray(out, dtype=np.float32)


# ----- fallback: pmap, 2 heads per core -----

def _per_core(X, mask, Wq, bq, Wk, bk, Wv, bv, proj):
    Q = X @ Wq + bq
    K = X @ Wk + bk
    V = X @ Wv + bv

    def split(x):  # [B,S,COLS] -> [B,HPC,S,HD]
        return x.reshape(B, S, HPC, HD).transpose(0, 2, 1, 3)

    out = _attn(split(Q), split(K), split(V), mask, proj)
    return out.transpose(0, 2, 1, 3).reshape(B, S, COLS)


_pmapped = None


def _run_pmap(X, mask, Wq, bq, Wk, bk, Wv, bv, proj):
    global _pmapped
    devs = jax.devices()[:N_CORES]
    if _pmapped is None:
        _pmapped = jax.pmap(_per_core, devices=devs)

    def shard_cols(W):
        return np.stack([np.asarray(W[:, i * COLS:(i + 1) * COLS]) for i in range(N_CORES)])

    def shard_bias(b):
        return np.stack([np.asarray(b[i * COLS:(i + 1) * COLS]) for i in range(N_CORES)])

    rep = lambda a: np.broadcast_to(np.asarray(a), (N_CORES,) + np.asarray(a).shape)

    outs = _pmapped(
        rep(X), rep(mask),
        shard_cols(Wq), shard_bias(bq),
        shard_cols(Wk), shard_bias(bk),
        shard_cols(Wv), shard_bias(bv),
        rep(proj),
    )
    outs = np.asarray(outs)  # [8,B,S,COLS]; core i -> output cols i*128:(i+1)*128
    return np.concatenate(list(outs), axis=-1).astype(np.float32)


_use_gspmd = True


def kernel(X, mask, Wq, bq, Wk, bk, Wv, bv, proj):
    global _use_gspmd
    if _use_gspmd:
        try:
            return _run_gspmd(X, mask, Wq, bq, Wk, bk, Wv, bv, proj)
        except Exception:
            _use_gspmd = False
    return _run_pmap(X, mask, Wq, bq, Wk, bk, Wv, bv, proj)


# revision 3
# speedup vs baseline: 4.8493x; 4.1669x over previous
"""Performer (FAVOR+) attention on 8 trn2 NeuronCores.

Sharding: tensor-parallel over the 16 heads. Primary path uses jit+GSPMD
with X replicated via a single device_put and the QKV projections
column-sharded (2 heads / core), so every op partitions head-locally with
no collectives. Falls back to an equivalent jax.pmap implementation if
the GSPMD path fails.
"""
import numpy as np
import jax
import jax.numpy as jnp
from jax.sharding import Mesh, NamedSharding, PartitionSpec as P

B, S, D = 4, 4096, 1024
H = 16
HD = 64          # head dim
M = 256          # nb random features
N_CORES = 8
HPC = H // N_CORES          # heads per core = 2
COLS = HPC * HD             # projection columns per core = 128


def _feat(x, proj, is_query):
    ratio = M ** -0.5
    x = x * (HD ** -0.25)
    u = jnp.einsum('bhsd,md->bhsm', x, proj)
    diag = 0.5 * jnp.sum(x * x, axis=-1, keepdims=True)
    if is_query:
        stab = jnp.max(u, axis=-1, keepdims=True)
    else:
        stab = jnp.max(u, axis=(-1, -2), keepdims=True)
    return ratio * (jnp.exp(u - diag - stab) + 1e-4)


def _attn(Q, K, V, mask, proj):
    # Q,K,V: [B,h,S,HD] for any number of heads h
    scale = HD ** -0.25
    m4 = mask[:, None, :, None]
    Qs = Q * scale
    Ks = K * scale * m4
    Vs = V * m4
    q_prime = _feat(Qs, proj, True)
    k_prime = _feat(Ks, proj, False)
    kv = jnp.einsum('bhsm,bhsd->bhmd', k_prime, Vs)
    z = 1.0 / (jnp.einsum('bhsm,bhm->bhs', q_prime, jnp.sum(k_prime, axis=2)) + 1e-6)
    return jnp.einsum('bhsm,bhmd->bhsd', q_prime, kv) * z[..., None]


# ----- primary: jit + GSPMD, full-shape math, head-sharded via weight cols -----

def _compute_full(X, mask, Wq, bq, Wk, bk, Wv, bv, proj):
    def split(x):  # [B,S,D] -> [B,H,S,HD]
        return x.reshape(B, S, H, HD).transpose(0, 2, 1, 3)

    Q = split(X @ Wq + bq)
    K = split(X @ Wk + bk)
    V = split(X @ Wv + bv)
    out = _attn(Q, K, V, mask, proj)          # [B,H,S,HD]
    return out.transpose(0, 2, 1, 3).reshape(B, S, D)


_gspmd = None


def _run_gspmd(X, mask, Wq, bq, Wk, bk, Wv, bv, proj):
    global _gspmd
    devs = jax.devices()[:N_CORES]
    mesh = Mesh(np.array(devs), ('x',))
    rep = NamedSharding(mesh, P())
    col = NamedSharding(mesh, P(None, 'x'))
    vec = NamedSharding(mesh, P('x'))
    seq = NamedSharding(mesh, P(None, 'x', None))  # X sharded over S: 1 host copy
    outsh = NamedSharding(mesh, P(None, None, 'x'))
    if _gspmd is None:
        _gspmd = jax.jit(
            _compute_full,
            in_shardings=(seq, rep, col, vec, col, vec, col, vec, rep),
            out_shardings=outsh,
        )
    args = (
        jax.device_put(np.asarray(X, np.float32), seq),
        jax.device_put(np.asarray(mask, np.float32), rep),
        jax.device_put(np.asarray(Wq, np.float32), col),
        jax.device_put(np.asarray(bq, np.float32), vec),
        jax.device_put(np.asarray(Wk, np.float32), col),
        jax.device_put(np.asarray(bk, np.float32), vec),
        jax.device_put(np.asarray(Wv, np.float32), col),
        jax.device_put(np.asarray(bv, np.float32), vec),
        jax.device_put(np.asarray(proj, np.float32), rep),
    )
    out = _gspmd(*args)
    return np.asarray(out, dtype=np.float32)


# ----- fallback: pmap, 2 heads per core -----

def _per_core(X, mask, Wq, bq, Wk, bk, Wv, bv, proj):
    Q = X @ Wq + bq
    K = X @ Wk + bk
    V = X @ Wv + bv

    def split(x):  # [B,S,COLS] -> [B,HPC,S,HD]
        return x.reshape(B, S, HPC, HD).transpose(0, 2, 1, 3)

    out = _attn(split(Q), split(K), split(V), mask, proj)
    return out.transpose(0, 2, 1, 3).reshape(B, S, COLS)


_pmapped = None


def _run_pmap(X, mask, Wq, bq, Wk, bk, Wv, bv, proj):
    global _pmapped
    devs = jax.devices()[:N_CORES]
    if _pmapped is None:
        _pmapped = jax.pmap(_per_core, devices=devs)

    def shard_cols(W):
        return np.stack([np.asarray(W[:, i * COLS:(i + 1) * COLS]) for i in range(N_CORES)])

    def shard_bias(b):
        return np.stack([np.asarray(b[i * COLS:(i + 1) * COLS]) for i in range(N_CORES)])

    rep = lambda a: np.broadcast_to(np.asarray(a), (N_CORES,) + np.asarray(a).shape)

    outs = _pmapped(
        rep(X), rep(mask),
        shard_cols(Wq), shard_bias(bq),
        shard_cols(Wk), shard_bias(bk),
        shard_cols(Wv), shard_bias(bv),
        rep(proj),
    )
    outs = np.asarray(outs)  # [8,B,S,COLS]; core i -> output cols i*128:(i+1)*128
    return np.concatenate(list(outs), axis=-1).astype(np.float32)


_use_gspmd = True


def kernel(X, mask, Wq, bq, Wk, bk, Wv, bv, proj):
    global _use_gspmd
    if _use_gspmd:
        try:
            return _run_gspmd(X, mask, Wq, bq, Wk, bk, Wv, bv, proj)
        except Exception:
            _use_gspmd = False
    return _run_pmap(X, mask, Wq, bq, Wk, bk, Wv, bv, proj)
